# revision 6
# baseline (speedup 1.0000x reference)
"""Crystal segment-norm kernel for 8 Trainium2 NeuronCores.

Strategy (fp16 end-to-end, packed groups-on-partition layout):
- Host pads each segment's atom rows to a multiple of G=8 (zero rows) and
  packs whole segments into fixed 8192-atom chunks (<=127 segments each,
  slot 127 = trash for padding). All 8 cores run the same compiled program
  (SPMD) on their own slice. x is converted to fp16 on the host.
- A device tile is [128 partitions = groups-of-8-atoms, 1024 = 8 atoms x
  128 feat]; each partition line is 2KB contiguous in DRAM so DMA runs at
  full bandwidth with 128 descriptors per 256KB tile.
- Pass 1 per tile: ACT squares the tile; DVE builds the group->segment
  one-hot mask via is_equal; 16 fp16 matmuls (one per atom slot, for x and
  x^2) accumulate per-segment [sum | sumsq] into PSUM across the chunk.
- Per chunk: mean/var/scale math on [128 segs, 128 feat] fp32 tiles:
  K = weight/(sqrt(max(((sumsq - mean*sum) + EPS)/(n-1), FLOOR)) + EPS),
  C = bias - mean*K, packed to fp16 kc = [K | C].
- Pass 2 per tile: gather per-group KC rows (matmul with mask^T), then
  out = x*K + C with two DVE tensor_tensor ops using a stride-0 broadcast
  of KC over the 8 atom slots. Output written fp16, host converts to fp32.
"""
import numpy as np

N = 1_000_000
F = 128
S = 16_384
EPS = 1e-6
VAR_FLOOR = 1e-7
NCORES = 8
G = 8                      # segment atom-count padding granularity
P = 128
TILE_ATOMS = P * G             # 1024 atoms per device tile
TPC = 8                    # tiles per chunk
CHUNK_ATOMS = TPC * TILE_ATOMS  # 8192
MAXSEG = 127               # real segments per chunk; slot 127 = trash
TRASH = 127


def _plan(index):
    """Pack segments into per-core chunk layouts. Returns per-core plans."""
    counts = np.bincount(index, minlength=S).astype(np.int64)
    seg_start = np.concatenate([[0], np.cumsum(counts)[:-1]])
    pad = ((counts + G - 1) // G) * G
    csum = np.cumsum(pad)
    total = int(csum[-1])
    bounds = [0]
    for c in range(1, NCORES):
        bounds.append(int(np.searchsorted(csum, total * c / NCORES)))
    bounds.append(S)

    plans = []
    for c in range(NCORES):
        segs = [s for s in range(bounds[c], bounds[c + 1]) if counts[s] > 0]
        chunks = []
        cur, cur_atoms = [], 0
        for s in segs:
            p = int(pad[s])
            assert p <= CHUNK_ATOMS
            if cur_atoms + p > CHUNK_ATOMS or len(cur) >= MAXSEG:
                chunks.append(cur)
                cur, cur_atoms = [], 0
            cur.append(s)
            cur_atoms += p
        if cur:
            chunks.append(cur)
        plans.append((chunks, counts, seg_start, pad))
    return plans


def _core_arrays(plan, nchunks, x):
    """Build xpad/gseg/rn/rn1 + row maps for one core."""
    chunks, counts, seg_start, pad = plan
    nat = nchunks * CHUNK_ATOMS
    ngrp = nat // G
    gseg = np.full(ngrp, TRASH, dtype=np.float32)
    rn = np.ones((nchunks, P), dtype=np.float32)
    rn1 = np.ones((nchunks, P), dtype=np.float32)

    seg_n, seg_src, seg_dst = [], [], []
    n1_dst = []  # dst rows of n==1 segments (host post-fix)
    for ci, segs in enumerate(chunks):
        off = ci * CHUNK_ATOMS
        for l, s in enumerate(segs):
            n = int(counts[s])
            p = int(pad[s])
            seg_n.append(n)
            seg_src.append(int(seg_start[s]))
            seg_dst.append(off)
            gseg[off // G:(off + p) // G] = l
            rn[ci, l] = 1.0 / n
            rn1[ci, l] = 1.0 / (n - 1) if n > 1 else 1.0
            if n == 1:
                n1_dst.append(off)
            off += p

    seg_n = np.array(seg_n, dtype=np.int64)
    seg_src = np.array(seg_src, dtype=np.int64)
    seg_dst = np.array(seg_dst, dtype=np.int64)
    tot = int(seg_n.sum())
    starts = np.concatenate([[0], np.cumsum(seg_n)[:-1]])
    local = np.arange(tot, dtype=np.int64) - np.repeat(starts, seg_n)
    src_rows = np.repeat(seg_src, seg_n) + local
    dst_rows = np.repeat(seg_dst, seg_n) + local

    xpad = np.zeros((nat, F), dtype=np.float16)
    xpad[dst_rows] = x[src_rows]
    # per-chunk group->slot ids, viewed [chunk, P, TPC] with g = t*P + p
    gsegv = gseg.reshape(nchunks, TPC, P).transpose(0, 2, 1)
    return {
        "xpad": xpad,
        "gseg": np.ascontiguousarray(gsegv).reshape(nchunks * P, TPC),
        "rn": np.ascontiguousarray(rn.reshape(nchunks, P).T),
        "rn1": np.ascontiguousarray(rn1.reshape(nchunks, P).T),
        "src_rows": src_rows,
        "dst_rows": dst_rows,
        "n1_dst": np.array(n1_dst, dtype=np.int64),
    }


def _consts(weight, bias):
    iota_row = np.tile(np.arange(P, dtype=np.float16), (P, 1))
    return {
        "ident": np.eye(P, dtype=np.float16),
        "iota": iota_row,
        "wb": np.tile(np.asarray(weight, dtype=np.float32), (P, 1)),
        "bb": np.tile(np.asarray(bias, dtype=np.float32), (P, 1)),
    }


def _build(nchunks):
    import concourse.tile as tile
    from concourse import bacc, mybir

    F32 = mybir.dt.float32
    F16 = mybir.dt.float16
    AF = mybir.ActivationFunctionType
    OP = mybir.AluOpType

    nat = nchunks * CHUNK_ATOMS
    ntiles = nat // TILE_ATOMS
    nc = bacc.Bacc("TRN2", target_bir_lowering=False, debug=False,
                   num_devices=NCORES)
    x_d = nc.dram_tensor("xpad", [nat, F], F16, kind="ExternalInput")
    out_d = nc.dram_tensor("out", [nat, F], F16, kind="ExternalOutput")
    gseg_d = nc.dram_tensor("gseg", [nchunks * P, TPC], F32,
                            kind="ExternalInput")
    rn_d = nc.dram_tensor("rn", [P, nchunks], F32, kind="ExternalInput")
    rn1_d = nc.dram_tensor("rn1", [P, nchunks], F32, kind="ExternalInput")
    id_d = nc.dram_tensor("ident", [P, P], F16, kind="ExternalInput")
    iota_d = nc.dram_tensor("iota", [P, P], F16, kind="ExternalInput")
    wb_d = nc.dram_tensor("wb", [P, P], F32, kind="ExternalInput")
    bb_d = nc.dram_tensor("bb", [P, P], F32, kind="ExternalInput")

    xv = x_d.ap().rearrange("(t p a) f -> t p (a f)", p=P, a=G)
    ov = out_d.ap().rearrange("(t p a) f -> t p (a f)", p=P, a=G)
    gsegv = gseg_d.ap().rearrange("(c p) t -> c p t", p=P)

    with tile.TileContext(nc) as tc:
        with (
            tc.tile_pool(name="consts", bufs=1) as cpool,
            tc.tile_pool(name="xx", bufs=24) as xxp,
            tc.tile_pool(name="sq", bufs=3) as sqp,
            tc.tile_pool(name="small", bufs=4) as smallp,
            tc.tile_pool(name="apool", bufs=2 * TPC + 6) as apool,
            tc.tile_pool(name="work", bufs=4) as workp,
            tc.tile_pool(name="kcp", bufs=2) as kcp,
            tc.tile_pool(name="stats", bufs=2) as statsp,
            tc.tile_pool(name="opool", bufs=4) as opool,
            tc.tile_pool(name="ps_seg", bufs=2, space="PSUM") as ps_seg,
            tc.tile_pool(name="ps_aux", bufs=2, space="PSUM") as ps_aux,
            tc.tile_pool(name="ps_kc", bufs=2, space="PSUM") as ps_kc,
        ):
            id_t = cpool.tile([P, P], F16)
            nc.sync.dma_start(out=id_t[:], in_=id_d.ap()[:, :])
            iota_t = cpool.tile([P, P], F16)
            nc.sync.dma_start(out=iota_t[:], in_=iota_d.ap()[:, :])
            wb_t = cpool.tile([P, P], F32)
            nc.sync.dma_start(out=wb_t[:], in_=wb_d.ap()[:, :])
            bb_t = cpool.tile([P, P], F32)
            nc.sync.dma_start(out=bb_t[:], in_=bb_d.ap()[:, :])
            rn_all = cpool.tile([P, nchunks], F32)
            nc.sync.dma_start(out=rn_all[:], in_=rn_d.ap()[:, :])
            rn1_all = cpool.tile([P, nchunks], F32)
            nc.sync.dma_start(out=rn1_all[:], in_=rn1_d.ap()[:, :])

            def pass2_tile(ctx, kc):
                """Expand per-segment KC to groups, normalize, store."""
                a_sb, xx3, tg = ctx
                kcg_ps = ps_kc.tile([P, 2 * P], F32, space="PSUM", tag="kcps")
                nc.tensor.matmul(
                    out=kcg_ps[:], lhsT=a_sb[:], rhs=kc[:],
                    start=True, stop=True,
                )
                kcg = workp.tile([P, 2 * P], F16, tag="kcg")
                nc.scalar.copy(out=kcg[:], in_=kcg_ps[:])
                o = opool.tile([P, TILE_ATOMS], F16, tag="o")
                o3 = o[:].rearrange("p (a f) -> p a f", f=P)
                kb = kcg[:, 0:P].unsqueeze(1).broadcast_to([P, G, P])
                cb = kcg[:, P:2 * P].unsqueeze(1).broadcast_to([P, G, P])
                nc.vector.tensor_tensor(out=o3[:], in0=xx3[:], in1=kb,
                                        op=OP.mult)
                add_eng = nc.gpsimd if tg % 3 == 0 else nc.vector
                add_eng.tensor_tensor(out=o3[:], in0=o3[:], in1=cb,
                                      op=OP.add)
                nc.sync.dma_start(out=ov[tg, :, :], in_=o[:])

            def pass1_tile(c, t, seg_ps, gseg_chunk, p2job):
                """Load tile, accumulate [sum|sumsq] into seg_ps; weave in a
                pass-2 job from an older chunk for engine overlap."""
                tg = c * TPC + t
                xx = xxp.tile([P, TILE_ATOMS], F16, tag="xx")
                nc.sync.dma_start(out=xx[:], in_=xv[tg, :, :])
                xx3 = xx[:].rearrange("p (a f) -> p a f", f=P)

                at = workp.tile([P, P], F16, tag="at")
                nc.vector.tensor_scalar(
                    out=at[:], in0=iota_t[:], scalar1=gseg_chunk[:, t:t + 1],
                    scalar2=None, op0=OP.is_equal,
                )
                xsq = sqp.tile([P, TILE_ATOMS], F16, tag="xsq")
                if tg % 3 == 1:
                    nc.gpsimd.tensor_tensor(out=xsq[:], in0=xx[:], in1=xx[:],
                                            op=OP.mult)
                else:
                    nc.scalar.activation(out=xsq[:], in_=xx[:], func=AF.Square)
                xsq3 = xsq[:].rearrange("p (a f) -> p a f", f=P)

                # One accumulation group per chunk for the whole seg_ps bank:
                # the first matmul's start=True lazily zeroes the full 2KB
                # zero region (covering both the sum and sumsq columns), so
                # every other matmul must use start=False.
                for a in range(G):
                    nc.tensor.matmul(
                        out=seg_ps[:, 0:P], lhsT=at[:], rhs=xx3[:, a, :],
                        start=(t == 0 and a == 0), stop=False,
                    )
                if p2job is not None:
                    pass2_tile(*p2job)
                for a in range(G):
                    nc.tensor.matmul(
                        out=seg_ps[:, P:2 * P], lhsT=at[:], rhs=xsq3[:, a, :],
                        start=False, stop=(t == TPC - 1 and a == G - 1),
                    )
                a_ps = ps_aux.tile([P, P], F16, space="PSUM", tag="aux")
                nc.tensor.transpose(out=a_ps[:], in_=at[:], identity=id_t[:])
                a_sb = apool.tile([P, P], F16, tag="a")
                nc.scalar.copy(out=a_sb[:], in_=a_ps[:])
                return (a_sb, xx3, tg)

            def stats_chunk(c, seg_ps):
                rn_t = rn_all[:, c:c + 1]
                rn1_t = rn1_all[:, c:c + 1]
                kc = kcp.tile([P, 2 * P], F16, tag="kc")
                mean_t = statsp.tile([P, P], F32, tag="mean")
                nc.vector.tensor_scalar(
                    out=mean_t[:], in0=seg_ps[:, 0:P], scalar1=rn_t,
                    scalar2=None, op0=OP.mult,
                )
                t1 = statsp.tile([P, P], F32, tag="t1")
                nc.vector.tensor_tensor(
                    out=t1[:], in0=mean_t[:], in1=seg_ps[:, 0:P], op=OP.mult,
                )
                t2 = statsp.tile([P, P], F32, tag="t2")
                nc.vector.tensor_tensor(
                    out=t2[:], in0=seg_ps[:, P:2 * P], in1=t1[:],
                    op=OP.subtract,
                )
                var_t = statsp.tile([P, P], F32, tag="var")
                nc.vector.tensor_scalar(
                    out=var_t[:], in0=t2[:], scalar1=float(EPS),
                    scalar2=rn1_t, op0=OP.add, op1=OP.mult,
                )
                nc.vector.tensor_scalar(
                    out=var_t[:], in0=var_t[:], scalar1=float(VAR_FLOOR),
                    scalar2=None, op0=OP.max,
                )
                std_t = statsp.tile([P, P], F32, tag="std")
                nc.scalar.activation(out=std_t[:], in_=var_t[:], func=AF.Sqrt)
                nc.scalar.activation(
                    out=std_t[:], in_=std_t[:], func=AF.Copy, bias=float(EPS),
                )
                rstd_t = statsp.tile([P, P], F32, tag="rstd")
                nc.vector.reciprocal(out=rstd_t[:], in_=std_t[:])
                k_t = statsp.tile([P, P], F32, tag="k")
                nc.vector.tensor_tensor(
                    out=k_t[:], in0=rstd_t[:], in1=wb_t[:], op=OP.mult,
                )
                nc.scalar.copy(out=kc[:, 0:P], in_=k_t[:])
                mk_t = statsp.tile([P, P], F32, tag="mk")
                nc.vector.tensor_tensor(
                    out=mk_t[:], in0=mean_t[:], in1=k_t[:], op=OP.mult,
                )
                c_t = statsp.tile([P, P], F32, tag="c")
                nc.vector.tensor_tensor(
                    out=c_t[:], in0=bb_t[:], in1=mk_t[:], op=OP.subtract,
                )
                nc.scalar.copy(out=kc[:, P:2 * P], in_=c_t[:])
                return kc

            LAG = 12
            p2q = []          # fifo of (ctx, chunk) awaiting pass2
            kc_by_chunk = {}
            for c in range(nchunks):
                seg_ps = ps_seg.tile([P, 2 * P], F32, space="PSUM", tag="seg")
                gseg_chunk = smallp.tile([P, TPC], F32, tag="gsegc")
                nc.sync.dma_start(out=gseg_chunk[:], in_=gsegv[c, :, :])
                for t in range(TPC):
                    job = None
                    if len(p2q) >= LAG:
                        ctx0, c0 = p2q.pop(0)
                        job = (ctx0, kc_by_chunk[c0])
                    ctx = pass1_tile(c, t, seg_ps, gseg_chunk, job)
                    p2q.append((ctx, c))
                kc_by_chunk[c] = stats_chunk(c, seg_ps)
            for ctx0, c0 in p2q:
                pass2_tile(ctx0, kc_by_chunk[c0])

    nc.compile()
    return nc


_BUILD_CACHE = {}


def kernel(target_fea, index, weight, bias):
    from concourse.bass_utils import run_bass_kernel_spmd

    x = np.asarray(target_fea, dtype=np.float32)
    idx = np.asarray(index, dtype=np.int64)
    plans = _plan(idx)
    nchunks = max(len(p[0]) for p in plans)
    consts = _consts(weight, bias)

    cores = [_core_arrays(p, nchunks, x) for p in plans]
    in_maps = []
    for ca in cores:
        m = {"xpad": ca["xpad"], "gseg": ca["gseg"], "rn": ca["rn"],
             "rn1": ca["rn1"]}
        m.update(consts)
        in_maps.append(m)

    if nchunks not in _BUILD_CACHE:
        _BUILD_CACHE[nchunks] = _build(nchunks)
    nc = _BUILD_CACHE[nchunks]

    res = run_bass_kernel_spmd(nc, in_maps, core_ids=list(range(NCORES)))

    out = np.empty((N, F), dtype=np.float32)
    bias_np = np.asarray(bias, dtype=np.float32)
    for c in range(NCORES):
        ca = cores[c]
        out[ca["src_rows"]] = res.results[c]["out"][ca["dst_rows"]]
        for d in ca["n1_dst"]:
            # n==1 segments: reference yields exactly bias
            src = ca["src_rows"][np.searchsorted(ca["dst_rows"], d)]
            out[src] = bias_np
    return out


# revision 12
# speedup vs baseline: 1.3228x; 1.3228x over previous
"""Crystal segment-norm kernel for 8 Trainium2 NeuronCores.

Strategy (fp16 end-to-end, packed groups-on-partition layout):
- Host pads each segment's atom rows to a multiple of G=8 (zero rows) and
  packs whole segments into fixed 8192-atom chunks (<=127 segments each,
  slot 127 = trash for padding). All 8 cores run the same compiled program
  (SPMD) on their own slice. x is converted to fp16 on the host.
- A device tile is [128 partitions = groups-of-8-atoms, 1024 = 8 atoms x
  128 feat]; each partition line is 2KB contiguous in DRAM so DMA runs at
  full bandwidth with 128 descriptors per 256KB tile.
- Pass 1 per tile: ACT squares the tile; DVE builds the group->segment
  one-hot mask via is_equal; 16 fp16 matmuls (one per atom slot, for x and
  x^2) accumulate per-segment [sum | sumsq] into PSUM across the chunk.
- Per chunk: mean/var/scale math on [128 segs, 128 feat] fp32 tiles:
  K = weight/(sqrt(max(((sumsq - mean*sum) + EPS)/(n-1), FLOOR)) + EPS),
  C = bias - mean*K, packed to fp16 kc = [K | C].
- Pass 2 per tile: gather per-group KC rows (matmul with mask^T), then
  out = x*K + C with two DVE tensor_tensor ops using a stride-0 broadcast
  of KC over the 8 atom slots. Output written fp16, host converts to fp32.
"""
import numpy as np

N = 1_000_000
F = 128
S = 16_384
EPS = 1e-6
VAR_FLOOR = 1e-7
NCORES = 8
G = 8                      # segment atom-count padding granularity
P = 128
TILE_ATOMS = P * G             # 1024 atoms per device tile
TPC = 8                    # tiles per chunk
CHUNK_ATOMS = TPC * TILE_ATOMS  # 8192
MAXSEG = 127               # real segments per chunk; slot 127 = trash
TRASH = 127


def _plan(index):
    """Pack segments into per-core chunk layouts. Returns per-core plans."""
    counts = np.bincount(index, minlength=S).astype(np.int64)
    seg_start = np.concatenate([[0], np.cumsum(counts)[:-1]])
    pad = ((counts + G - 1) // G) * G
    csum = np.cumsum(pad)
    total = int(csum[-1])
    bounds = [0]
    for c in range(1, NCORES):
        bounds.append(int(np.searchsorted(csum, total * c / NCORES)))
    bounds.append(S)

    plans = []
    for c in range(NCORES):
        segs = [s for s in range(bounds[c], bounds[c + 1]) if counts[s] > 0]
        chunks = []
        cur, cur_atoms = [], 0
        for s in segs:
            p = int(pad[s])
            assert p <= CHUNK_ATOMS
            if cur_atoms + p > CHUNK_ATOMS or len(cur) >= MAXSEG:
                chunks.append(cur)
                cur, cur_atoms = [], 0
            cur.append(s)
            cur_atoms += p
        if cur:
            chunks.append(cur)
        plans.append((chunks, counts, seg_start, pad))
    return plans


def _core_arrays(plan, nchunks, x):
    """Build xpad/gseg/rn/rn1 + row maps for one core."""
    chunks, counts, seg_start, pad = plan
    nat = nchunks * CHUNK_ATOMS
    ngrp = nat // G
    gseg = np.full(ngrp, TRASH, dtype=np.float32)
    rn = np.ones((nchunks, P), dtype=np.float32)
    rn1 = np.ones((nchunks, P), dtype=np.float32)

    seg_n, seg_src, seg_dst = [], [], []
    n1_dst = []  # dst rows of n==1 segments (host post-fix)
    for ci, segs in enumerate(chunks):
        off = ci * CHUNK_ATOMS
        for l, s in enumerate(segs):
            n = int(counts[s])
            p = int(pad[s])
            seg_n.append(n)
            seg_src.append(int(seg_start[s]))
            seg_dst.append(off)
            gseg[off // G:(off + p) // G] = l
            rn[ci, l] = 1.0 / n
            rn1[ci, l] = 1.0 / (n - 1) if n > 1 else 1.0
            if n == 1:
                n1_dst.append(off)
            off += p

    seg_n = np.array(seg_n, dtype=np.int64)
    seg_src = np.array(seg_src, dtype=np.int64)
    seg_dst = np.array(seg_dst, dtype=np.int64)
    tot = int(seg_n.sum())
    starts = np.concatenate([[0], np.cumsum(seg_n)[:-1]])
    local = np.arange(tot, dtype=np.int64) - np.repeat(starts, seg_n)
    src_rows = np.repeat(seg_src, seg_n) + local
    dst_rows = np.repeat(seg_dst, seg_n) + local

    xpad = np.zeros((nat, F), dtype=np.float16)
    xpad[dst_rows] = x[src_rows]
    # per-chunk group->slot ids, viewed [chunk, P, TPC] with g = t*P + p
    gsegv = gseg.reshape(nchunks, TPC, P).transpose(0, 2, 1)
    return {
        "xpad": xpad,
        "gseg": np.ascontiguousarray(gsegv).reshape(nchunks * P, TPC),
        "rn": np.ascontiguousarray(rn.reshape(nchunks, P).T),
        "rn1": np.ascontiguousarray(rn1.reshape(nchunks, P).T),
        "src_rows": src_rows,
        "dst_rows": dst_rows,
        "n1_dst": np.array(n1_dst, dtype=np.int64),
    }


def _consts(weight, bias):
    iota_row = np.tile(np.arange(P, dtype=np.float16), (P, 1))
    return {
        "ident": np.eye(P, dtype=np.float16),
        "iota": iota_row,
        "wb": np.tile(np.asarray(weight, dtype=np.float32), (P, 1)),
        "bb": np.tile(np.asarray(bias, dtype=np.float32), (P, 1)),
    }


def _build(nchunks):
    import concourse.tile as tile
    from concourse import bacc, mybir

    F32 = mybir.dt.float32
    F16 = mybir.dt.float16
    AF = mybir.ActivationFunctionType
    OP = mybir.AluOpType

    nat = nchunks * CHUNK_ATOMS
    ntiles = nat // TILE_ATOMS
    nc = bacc.Bacc("TRN2", target_bir_lowering=False, debug=False,
                   num_devices=NCORES)
    x_d = nc.dram_tensor("xpad", [nat, F], F16, kind="ExternalInput")
    out_d = nc.dram_tensor("out", [nat, F], F16, kind="ExternalOutput")
    gseg_d = nc.dram_tensor("gseg", [nchunks * P, TPC], F32,
                            kind="ExternalInput")
    rn_d = nc.dram_tensor("rn", [P, nchunks], F32, kind="ExternalInput")
    rn1_d = nc.dram_tensor("rn1", [P, nchunks], F32, kind="ExternalInput")
    id_d = nc.dram_tensor("ident", [P, P], F16, kind="ExternalInput")
    iota_d = nc.dram_tensor("iota", [P, P], F16, kind="ExternalInput")
    wb_d = nc.dram_tensor("wb", [P, P], F32, kind="ExternalInput")
    bb_d = nc.dram_tensor("bb", [P, P], F32, kind="ExternalInput")

    # double-tile views: one DMA covers two 1024-atom tiles (2 runs of 2KB
    # per partition) to halve trigger/instruction counts
    xv = x_d.ap().rearrange("(d t p a) f -> d p t (a f)", t=2, p=P, a=G)
    ov = out_d.ap().rearrange("(d t p a) f -> d p t (a f)", t=2, p=P, a=G)
    gsegv = gseg_d.ap().rearrange("(c p) t -> c p t", p=P)

    with tile.TileContext(nc) as tc:
        with (
            tc.tile_pool(name="consts", bufs=1) as cpool,
            tc.tile_pool(name="xx", bufs=12) as xxp,
            tc.tile_pool(name="sq", bufs=2) as sqp,
            tc.tile_pool(name="small", bufs=4) as smallp,
            tc.tile_pool(name="apool", bufs=2 * TPC + 6) as apool,
            tc.tile_pool(name="work", bufs=4) as workp,
            tc.tile_pool(name="kcp", bufs=2) as kcp,
            tc.tile_pool(name="stats", bufs=2) as statsp,
            tc.tile_pool(name="opool", bufs=4) as opool,
            tc.tile_pool(name="ps_seg", bufs=2, space="PSUM") as ps_seg,
            tc.tile_pool(name="ps_aux", bufs=2, space="PSUM") as ps_aux,
            tc.tile_pool(name="ps_kc", bufs=2, space="PSUM") as ps_kc,
        ):
            id_t = cpool.tile([P, P], F16)
            nc.sync.dma_start(out=id_t[:], in_=id_d.ap()[:, :])
            iota_t = cpool.tile([P, P], F16)
            nc.sync.dma_start(out=iota_t[:], in_=iota_d.ap()[:, :])
            wb_t = cpool.tile([P, P], F32)
            nc.sync.dma_start(out=wb_t[:], in_=wb_d.ap()[:, :])
            bb_t = cpool.tile([P, P], F32)
            nc.sync.dma_start(out=bb_t[:], in_=bb_d.ap()[:, :])
            rn_all = cpool.tile([P, nchunks], F32)
            nc.sync.dma_start(out=rn_all[:], in_=rn_d.ap()[:, :])
            rn1_all = cpool.tile([P, nchunks], F32)
            nc.sync.dma_start(out=rn1_all[:], in_=rn1_d.ap()[:, :])

            def pass2_half(a_sb, xx4, o4, h, kc):
                """Expand per-segment KC to groups, normalize one half."""
                kcg_ps = ps_kc.tile([P, 2 * P], F32, space="PSUM", tag="kcps")
                nc.tensor.matmul(
                    out=kcg_ps[:], lhsT=a_sb[:], rhs=kc[:],
                    start=True, stop=True,
                )
                kcg = workp.tile([P, 2 * P], F16, tag="kcg")
                nc.scalar.copy(out=kcg[:], in_=kcg_ps[:])
                kb = kcg[:, 0:P].unsqueeze(1).broadcast_to([P, G, P])
                cb = kcg[:, P:2 * P].unsqueeze(1).broadcast_to([P, G, P])
                nc.vector.tensor_tensor(out=o4[:, h, :, :], in0=xx4[:, h, :, :],
                                        in1=kb, op=OP.mult)
                nc.vector.tensor_tensor(out=o4[:, h, :, :], in0=o4[:, h, :, :],
                                        in1=cb, op=OP.add)

            def pass2_dt(ctx, kc):
                a_sbs, xx4, dg = ctx
                o = opool.tile([P, 2 * TILE_ATOMS], F16, tag="o")
                o4 = o[:].rearrange("p (t a f) -> p t a f", a=G, f=P)
                for h in range(2):
                    pass2_half(a_sbs[h], xx4, o4, h, kc)
                nc.gpsimd.dma_start(out=ov[dg, :, :, :], in_=o[:].rearrange("p (t af) -> p t af", t=2))

            def pass1_half(c, t, seg_ps, gseg_chunk, xx4, xsq4, h):
                at = workp.tile([P, P], F16, tag="at")
                nc.vector.tensor_scalar(
                    out=at[:], in0=iota_t[:], scalar1=gseg_chunk[:, t:t + 1],
                    scalar2=None, op0=OP.is_equal,
                )
                # One accumulation group per chunk for the whole seg_ps bank:
                # the first matmul's start=True lazily zeroes the full 2KB
                # zero region (covering both the sum and sumsq columns), so
                # every other matmul must use start=False.
                for a in range(G):
                    nc.tensor.matmul(
                        out=seg_ps[:, 0:P], lhsT=at[:], rhs=xx4[:, h, a, :],
                        start=(t == 0 and a == 0), stop=False,
                    )
                for a in range(G):
                    nc.tensor.matmul(
                        out=seg_ps[:, P:2 * P], lhsT=at[:], rhs=xsq4[:, h, a, :],
                        start=False, stop=(t == TPC - 1 and a == G - 1),
                    )
                a_ps = ps_aux.tile([P, P], F16, space="PSUM", tag="aux")
                nc.tensor.transpose(out=a_ps[:], in_=at[:], identity=id_t[:])
                a_sb = apool.tile([P, P], F16, tag="a")
                nc.scalar.copy(out=a_sb[:], in_=a_ps[:])
                return a_sb

            def pass1_dt(c, d, seg_ps, gseg_chunk, p2job):
                """Load a double tile, accumulate [sum|sumsq] into seg_ps;
                weave in a pass-2 job from an older chunk for overlap."""
                dg = c * (TPC // 2) + d
                xx = xxp.tile([P, 2 * TILE_ATOMS], F16, tag="xx")
                nc.sync.dma_start(out=xx[:].rearrange("p (t af) -> p t af", t=2), in_=xv[dg, :, :, :])
                xx4 = xx[:].rearrange("p (t a f) -> p t a f", a=G, f=P)
                xsq = sqp.tile([P, 2 * TILE_ATOMS], F16, tag="xsq")
                nc.scalar.activation(out=xsq[:], in_=xx[:], func=AF.Square)
                xsq4 = xsq[:].rearrange("p (t a f) -> p t a f", a=G, f=P)

                a_sbs = []
                for h in range(2):
                    a_sbs.append(
                        pass1_half(c, 2 * d + h, seg_ps, gseg_chunk, xx4,
                                   xsq4, h))
                    if h == 0 and p2job is not None:
                        pass2_dt(*p2job)
                return (a_sbs, xx4, dg)

            def stats_chunk(c, seg_ps):
                rn_t = rn_all[:, c:c + 1]
                rn1_t = rn1_all[:, c:c + 1]
                kc = kcp.tile([P, 2 * P], F16, tag="kc")
                mean_t = statsp.tile([P, P], F32, tag="mean")
                nc.vector.tensor_scalar(
                    out=mean_t[:], in0=seg_ps[:, 0:P], scalar1=rn_t,
                    scalar2=None, op0=OP.mult,
                )
                t1 = statsp.tile([P, P], F32, tag="t1")
                nc.vector.tensor_tensor(
                    out=t1[:], in0=mean_t[:], in1=seg_ps[:, 0:P], op=OP.mult,
                )
                t2 = statsp.tile([P, P], F32, tag="t2")
                nc.vector.tensor_tensor(
                    out=t2[:], in0=seg_ps[:, P:2 * P], in1=t1[:],
                    op=OP.subtract,
                )
                var_t = statsp.tile([P, P], F32, tag="var")
                nc.vector.tensor_scalar(
                    out=var_t[:], in0=t2[:], scalar1=float(EPS),
                    scalar2=rn1_t, op0=OP.add, op1=OP.mult,
                )
                nc.vector.tensor_scalar(
                    out=var_t[:], in0=var_t[:], scalar1=float(VAR_FLOOR),
                    scalar2=None, op0=OP.max,
                )
                std_t = statsp.tile([P, P], F32, tag="std")
                nc.scalar.activation(out=std_t[:], in_=var_t[:], func=AF.Sqrt)
                nc.scalar.activation(
                    out=std_t[:], in_=std_t[:], func=AF.Copy, bias=float(EPS),
                )
                rstd_t = statsp.tile([P, P], F32, tag="rstd")
                nc.vector.reciprocal(out=rstd_t[:], in_=std_t[:])
                k_t = statsp.tile([P, P], F32, tag="k")
                nc.vector.tensor_tensor(
                    out=k_t[:], in0=rstd_t[:], in1=wb_t[:], op=OP.mult,
                )
                nc.scalar.copy(out=kc[:, 0:P], in_=k_t[:])
                mk_t = statsp.tile([P, P], F32, tag="mk")
                nc.vector.tensor_tensor(
                    out=mk_t[:], in0=mean_t[:], in1=k_t[:], op=OP.mult,
                )
                c_t = statsp.tile([P, P], F32, tag="c")
                nc.vector.tensor_tensor(
                    out=c_t[:], in0=bb_t[:], in1=mk_t[:], op=OP.subtract,
                )
                nc.scalar.copy(out=kc[:, P:2 * P], in_=c_t[:])
                return kc

            LAG = 6
            p2q = []          # fifo of (ctx, chunk) awaiting pass2
            kc_by_chunk = {}
            for c in range(nchunks):
                seg_ps = ps_seg.tile([P, 2 * P], F32, space="PSUM", tag="seg")
                gseg_chunk = smallp.tile([P, TPC], F32, tag="gsegc")
                nc.sync.dma_start(out=gseg_chunk[:], in_=gsegv[c, :, :])
                for d in range(TPC // 2):
                    job = None
                    if len(p2q) >= LAG:
                        ctx0, c0 = p2q.pop(0)
                        job = (ctx0, kc_by_chunk[c0])
                    ctx = pass1_dt(c, d, seg_ps, gseg_chunk, job)
                    p2q.append((ctx, c))
                kc_by_chunk[c] = stats_chunk(c, seg_ps)
            for ctx0, c0 in p2q:
                pass2_dt(ctx0, kc_by_chunk[c0])

    nc.compile()
    return nc


_BUILD_CACHE = {}


def kernel(target_fea, index, weight, bias):
    from concourse.bass_utils import run_bass_kernel_spmd

    x = np.asarray(target_fea, dtype=np.float32)
    idx = np.asarray(index, dtype=np.int64)
    plans = _plan(idx)
    nchunks = max(len(p[0]) for p in plans)
    consts = _consts(weight, bias)

    cores = [_core_arrays(p, nchunks, x) for p in plans]
    in_maps = []
    for ca in cores:
        m = {"xpad": ca["xpad"], "gseg": ca["gseg"], "rn": ca["rn"],
             "rn1": ca["rn1"]}
        m.update(consts)
        in_maps.append(m)

    if nchunks not in _BUILD_CACHE:
        _BUILD_CACHE[nchunks] = _build(nchunks)
    nc = _BUILD_CACHE[nchunks]

    res = run_bass_kernel_spmd(nc, in_maps, core_ids=list(range(NCORES)))

    out = np.empty((N, F), dtype=np.float32)
    bias_np = np.asarray(bias, dtype=np.float32)
    for c in range(NCORES):
        ca = cores[c]
        out[ca["src_rows"]] = res.results[c]["out"][ca["dst_rows"]]
        for d in ca["n1_dst"]:
            # n==1 segments: reference yields exactly bias
            src = ca["src_rows"][np.searchsorted(ca["dst_rows"], d)]
            out[src] = bias_np
    return out


# revision 15
# speedup vs baseline: 1.3515x; 1.0217x over previous
"""Crystal segment-norm kernel for 8 Trainium2 NeuronCores.

Strategy (fp16 end-to-end, packed groups-on-partition layout):
- Host pads each segment's atom rows to a multiple of G=8 (zero rows) and
  packs whole segments into fixed 8192-atom chunks (<=127 segments each,
  slot 127 = trash for padding). All 8 cores run the same compiled program
  (SPMD) on their own slice. x is converted to fp16 on the host.
- A device tile is [128 partitions = groups-of-8-atoms, 1024 = 8 atoms x
  128 feat]; each partition line is 2KB contiguous in DRAM so DMA runs at
  full bandwidth with 128 descriptors per 256KB tile.
- Pass 1 per tile: ACT squares the tile; DVE builds the group->segment
  one-hot mask via is_equal; 16 fp16 matmuls (one per atom slot, for x and
  x^2) accumulate per-segment [sum | sumsq] into PSUM across the chunk.
- Per chunk: mean/var/scale math on [128 segs, 128 feat] fp32 tiles:
  K = weight/(sqrt(max(((sumsq - mean*sum) + EPS)/(n-1), FLOOR)) + EPS),
  C = bias - mean*K, packed to fp16 kc = [K | C].
- Pass 2 per tile: gather per-group KC rows (matmul with mask^T), then
  out = x*K + C with two DVE tensor_tensor ops using a stride-0 broadcast
  of KC over the 8 atom slots. Output written fp16, host converts to fp32.
"""
import numpy as np

N = 1_000_000
F = 128
S = 16_384
EPS = 1e-6
VAR_FLOOR = 1e-7
NCORES = 8
G = 8                      # segment atom-count padding granularity
P = 128
TILE_ATOMS = P * G             # 1024 atoms per device tile
TPC = 8                    # tiles per chunk
CHUNK_ATOMS = TPC * TILE_ATOMS  # 8192
MAXSEG = 127               # real segments per chunk; slot 127 = trash
TRASH = 127


def _plan(index):
    """Pack segments into per-core chunk layouts. Returns per-core plans."""
    counts = np.bincount(index, minlength=S).astype(np.int64)
    seg_start = np.concatenate([[0], np.cumsum(counts)[:-1]])
    pad = ((counts + G - 1) // G) * G
    csum = np.cumsum(pad)
    total = int(csum[-1])
    bounds = [0]
    for c in range(1, NCORES):
        bounds.append(int(np.searchsorted(csum, total * c / NCORES)))
    bounds.append(S)

    plans = []
    for c in range(NCORES):
        segs = [s for s in range(bounds[c], bounds[c + 1]) if counts[s] > 0]
        chunks = []
        cur, cur_atoms = [], 0
        for s in segs:
            p = int(pad[s])
            assert p <= CHUNK_ATOMS
            if cur_atoms + p > CHUNK_ATOMS or len(cur) >= MAXSEG:
                chunks.append(cur)
                cur, cur_atoms = [], 0
            cur.append(s)
            cur_atoms += p
        if cur:
            chunks.append(cur)
        plans.append((chunks, counts, seg_start, pad))
    return plans


def _core_arrays(plan, nchunks, x):
    """Build xpad/gseg/rn/rn1 + row maps for one core."""
    chunks, counts, seg_start, pad = plan
    nat = nchunks * CHUNK_ATOMS
    ngrp = nat // G
    gseg = np.full(ngrp, TRASH, dtype=np.float32)
    rn = np.ones((nchunks, P), dtype=np.float32)
    rn1 = np.ones((nchunks, P), dtype=np.float32)

    seg_n, seg_src, seg_dst = [], [], []
    n1_dst = []  # dst rows of n==1 segments (host post-fix)
    for ci, segs in enumerate(chunks):
        off = ci * CHUNK_ATOMS
        for l, s in enumerate(segs):
            n = int(counts[s])
            p = int(pad[s])
            seg_n.append(n)
            seg_src.append(int(seg_start[s]))
            seg_dst.append(off)
            gseg[off // G:(off + p) // G] = l
            rn[ci, l] = 1.0 / n
            rn1[ci, l] = 1.0 / (n - 1) if n > 1 else 1.0
            if n == 1:
                n1_dst.append(off)
            off += p

    seg_n = np.array(seg_n, dtype=np.int64)
    seg_src = np.array(seg_src, dtype=np.int64)
    seg_dst = np.array(seg_dst, dtype=np.int64)
    tot = int(seg_n.sum())
    starts = np.concatenate([[0], np.cumsum(seg_n)[:-1]])
    local = np.arange(tot, dtype=np.int64) - np.repeat(starts, seg_n)
    src_rows = np.repeat(seg_src, seg_n) + local
    dst_rows = np.repeat(seg_dst, seg_n) + local

    xpad = np.zeros((nat, F), dtype=np.float16)
    xpad[dst_rows] = x[src_rows]
    # group->slot ids as [P, nchunks*TPC] with g = c*CHUNK + t*P + p
    gsegv = gseg.reshape(nchunks, TPC, P).transpose(2, 0, 1)
    return {
        "xpad": xpad,
        "gseg": np.ascontiguousarray(gsegv).reshape(P, nchunks * TPC),
        "rn": np.ascontiguousarray(rn.reshape(nchunks, P).T),
        "rn1": np.ascontiguousarray(rn1.reshape(nchunks, P).T),
        "src_rows": src_rows,
        "dst_rows": dst_rows,
        "n1_dst": np.array(n1_dst, dtype=np.int64),
    }


def _consts(weight, bias):
    iota_row = np.tile(np.arange(P, dtype=np.float16), (P, 1))
    return {
        "ident": np.eye(P, dtype=np.float16),
        "iota": iota_row,
        "wb": np.tile(np.asarray(weight, dtype=np.float32), (P, 1)),
        "bb": np.tile(np.asarray(bias, dtype=np.float32), (P, 1)),
    }


def _build(nchunks):
    import concourse.tile as tile
    from concourse import bacc, mybir

    F32 = mybir.dt.float32
    F16 = mybir.dt.float16
    AF = mybir.ActivationFunctionType
    OP = mybir.AluOpType

    nat = nchunks * CHUNK_ATOMS
    ntiles = nat // TILE_ATOMS
    nc = bacc.Bacc("TRN2", target_bir_lowering=False, debug=False,
                   num_devices=NCORES)
    x_d = nc.dram_tensor("xpad", [nat, F], F16, kind="ExternalInput")
    out_d = nc.dram_tensor("out", [nat, F], F16, kind="ExternalOutput")
    gseg_d = nc.dram_tensor("gseg", [P, nchunks * TPC], F32,
                            kind="ExternalInput")
    rn_d = nc.dram_tensor("rn", [P, nchunks], F32, kind="ExternalInput")
    rn1_d = nc.dram_tensor("rn1", [P, nchunks], F32, kind="ExternalInput")
    id_d = nc.dram_tensor("ident", [P, P], F16, kind="ExternalInput")
    iota_d = nc.dram_tensor("iota", [P, P], F16, kind="ExternalInput")
    wb_d = nc.dram_tensor("wb", [P, P], F32, kind="ExternalInput")
    bb_d = nc.dram_tensor("bb", [P, P], F32, kind="ExternalInput")

    # double-tile views: one DMA covers two 1024-atom tiles (2 runs of 2KB
    # per partition) to halve trigger/instruction counts
    xv = x_d.ap().rearrange("(d t p a) f -> d p t (a f)", t=2, p=P, a=G)
    ov = out_d.ap().rearrange("(d t p a) f -> d p t (a f)", t=2, p=P, a=G)

    with tile.TileContext(nc) as tc:
        with (
            tc.tile_pool(name="consts", bufs=1) as cpool,
            tc.tile_pool(name="xx", bufs=12) as xxp,
            tc.tile_pool(name="sq", bufs=2) as sqp,
            tc.tile_pool(name="small", bufs=4) as smallp,
            tc.tile_pool(name="apool", bufs=12) as apool,
            tc.tile_pool(name="work", bufs=4) as workp,
            tc.tile_pool(name="kcp", bufs=2) as kcp,
            tc.tile_pool(name="stats", bufs=2) as statsp,
            tc.tile_pool(name="opool", bufs=4) as opool,
            tc.tile_pool(name="ps_seg", bufs=2, space="PSUM") as ps_seg,
            tc.tile_pool(name="ps_aux", bufs=2, space="PSUM") as ps_aux,
            tc.tile_pool(name="ps_kc", bufs=2, space="PSUM") as ps_kc,
        ):
            id_t = cpool.tile([P, P], F16)
            nc.sync.dma_start(out=id_t[:], in_=id_d.ap()[:, :])
            iota_t = cpool.tile([P, P], F16)
            nc.sync.dma_start(out=iota_t[:], in_=iota_d.ap()[:, :])
            wb_t = cpool.tile([P, P], F32)
            nc.sync.dma_start(out=wb_t[:], in_=wb_d.ap()[:, :])
            bb_t = cpool.tile([P, P], F32)
            nc.sync.dma_start(out=bb_t[:], in_=bb_d.ap()[:, :])
            rn_all = cpool.tile([P, nchunks], F32)
            nc.sync.dma_start(out=rn_all[:], in_=rn_d.ap()[:, :])
            rn1_all = cpool.tile([P, nchunks], F32)
            nc.sync.dma_start(out=rn1_all[:], in_=rn1_d.ap()[:, :])
            gseg_all = cpool.tile([P, nchunks * TPC], F32)
            nc.sync.dma_start(out=gseg_all[:], in_=gseg_d.ap()[:, :])

            def pass2_dt(ctx, kc):
                """Expand per-segment KC to groups, normalize, store a DT.
                Both halves' gathers share one PSUM bank (start=True only on
                the first; lazy-zero covers the second's region) so a single
                ACT copy moves both to SBUF."""
                a_sb2, xx4, dg = ctx
                kc_ps = ps_kc.tile([P, 4 * P], F32, space="PSUM", tag="kcps")
                for h in range(2):
                    nc.tensor.matmul(
                        out=kc_ps[:, 2 * h * P:2 * (h + 1) * P],
                        lhsT=a_sb2[:, h * P:(h + 1) * P], rhs=kc[:],
                        start=(h == 0), stop=(h == 1),
                    )
                kcg = workp.tile([P, 4 * P], F16, tag="kcg")
                nc.scalar.copy(out=kcg[:], in_=kc_ps[:])
                o = opool.tile([P, 2 * TILE_ATOMS], F16, tag="o")
                o4 = o[:].rearrange("p (t a f) -> p t a f", a=G, f=P)
                for h in range(2):
                    kb = kcg[:, 2 * h * P:(2 * h + 1) * P]
                    cb = kcg[:, (2 * h + 1) * P:2 * (h + 1) * P]
                    kb = kb.unsqueeze(1).broadcast_to([P, G, P])
                    cb = cb.unsqueeze(1).broadcast_to([P, G, P])
                    nc.vector.tensor_tensor(out=o4[:, h, :, :],
                                            in0=xx4[:, h, :, :],
                                            in1=kb, op=OP.mult)
                    nc.vector.tensor_tensor(out=o4[:, h, :, :],
                                            in0=o4[:, h, :, :],
                                            in1=cb, op=OP.add)
                nc.gpsimd.dma_start(out=ov[dg, :, :, :], in_=o[:].rearrange("p (t af) -> p t af", t=2))

            def pass1_half(c, t, seg_ps, gseg_chunk, xx4, xsq4, h):
                at = workp.tile([P, P], F16, tag="at")
                nc.vector.tensor_scalar(
                    out=at[:], in0=iota_t[:], scalar1=gseg_chunk[:, t:t + 1],
                    scalar2=None, op0=OP.is_equal,
                )
                # One accumulation group per chunk for the whole seg_ps bank:
                # the first matmul's start=True lazily zeroes the full 2KB
                # zero region (covering both the sum and sumsq columns), so
                # every other matmul must use start=False.
                for a in range(G):
                    nc.tensor.matmul(
                        out=seg_ps[:, 0:P], lhsT=at[:], rhs=xx4[:, h, a, :],
                        start=(t == 0 and a == 0), stop=False,
                    )
                for a in range(G):
                    nc.tensor.matmul(
                        out=seg_ps[:, P:2 * P], lhsT=at[:], rhs=xsq4[:, h, a, :],
                        start=False, stop=(t == TPC - 1 and a == G - 1),
                    )
                return at

            def pass1_dt(c, d, seg_ps, gseg_chunk, p2job):
                """Load a double tile, accumulate [sum|sumsq] into seg_ps;
                weave in a pass-2 job from an older chunk for overlap."""
                dg = c * (TPC // 2) + d
                xx = xxp.tile([P, 2 * TILE_ATOMS], F16, tag="xx")
                nc.sync.dma_start(out=xx[:].rearrange("p (t af) -> p t af", t=2), in_=xv[dg, :, :, :])
                xx4 = xx[:].rearrange("p (t a f) -> p t a f", a=G, f=P)
                xsq = sqp.tile([P, 2 * TILE_ATOMS], F16, tag="xsq")
                nc.scalar.activation(out=xsq[:], in_=xx[:], func=AF.Square)
                xsq4 = xsq[:].rearrange("p (t a f) -> p t a f", a=G, f=P)

                ats = []
                for h in range(2):
                    ats.append(
                        pass1_half(c, 2 * d + h, seg_ps, gseg_chunk, xx4,
                                   xsq4, h))
                    if h == 0 and p2job is not None:
                        pass2_dt(*p2job)
                # both mask transposes share one PSUM bank -> one ACT copy
                a_ps = ps_aux.tile([P, 2 * P], F16, space="PSUM", tag="aux")
                for h in range(2):
                    nc.tensor.matmul(
                        out=a_ps[:, h * P:(h + 1) * P], lhsT=ats[h],
                        rhs=id_t[:], is_transpose=True,
                        start=(h == 0), stop=(h == 1),
                    )
                a_sb2 = apool.tile([P, 2 * P], F16, tag="a")
                nc.scalar.copy(out=a_sb2[:], in_=a_ps[:])
                return (a_sb2, xx4, dg)

            def stats_chunk(c, seg_ps):
                rn_t = rn_all[:, c:c + 1]
                rn1_t = rn1_all[:, c:c + 1]
                kc = kcp.tile([P, 2 * P], F16, tag="kc")
                mean_t = statsp.tile([P, P], F32, tag="mean")
                nc.vector.tensor_scalar(
                    out=mean_t[:], in0=seg_ps[:, 0:P], scalar1=rn_t,
                    scalar2=None, op0=OP.mult,
                )
                t1 = statsp.tile([P, P], F32, tag="t1")
                nc.vector.tensor_tensor(
                    out=t1[:], in0=mean_t[:], in1=seg_ps[:, 0:P], op=OP.mult,
                )
                t2 = statsp.tile([P, P], F32, tag="t2")
                nc.vector.tensor_tensor(
                    out=t2[:], in0=seg_ps[:, P:2 * P], in1=t1[:],
                    op=OP.subtract,
                )
                var_t = statsp.tile([P, P], F32, tag="var")
                nc.vector.tensor_scalar(
                    out=var_t[:], in0=t2[:], scalar1=float(EPS),
                    scalar2=rn1_t, op0=OP.add, op1=OP.mult,
                )
                nc.vector.tensor_scalar(
                    out=var_t[:], in0=var_t[:], scalar1=float(VAR_FLOOR),
                    scalar2=None, op0=OP.max,
                )
                std_t = statsp.tile([P, P], F32, tag="std")
                nc.scalar.activation(out=std_t[:], in_=var_t[:], func=AF.Sqrt)
                nc.scalar.activation(
                    out=std_t[:], in_=std_t[:], func=AF.Copy, bias=float(EPS),
                )
                rstd_t = statsp.tile([P, P], F32, tag="rstd")
                nc.vector.reciprocal_approx_fast(out=rstd_t[:], in_=std_t[:])
                k_t = statsp.tile([P, P], F32, tag="k")
                nc.vector.tensor_tensor(
                    out=k_t[:], in0=rstd_t[:], in1=wb_t[:], op=OP.mult,
                )
                nc.scalar.copy(out=kc[:, 0:P], in_=k_t[:])
                mk_t = statsp.tile([P, P], F32, tag="mk")
                nc.vector.tensor_tensor(
                    out=mk_t[:], in0=mean_t[:], in1=k_t[:], op=OP.mult,
                )
                c_t = statsp.tile([P, P], F32, tag="c")
                nc.vector.tensor_tensor(
                    out=c_t[:], in0=bb_t[:], in1=mk_t[:], op=OP.subtract,
                )
                nc.scalar.copy(out=kc[:, P:2 * P], in_=c_t[:])
                return kc

            LAG = 6
            p2q = []          # fifo of (ctx, chunk) awaiting pass2
            kc_by_chunk = {}
            for c in range(nchunks):
                seg_ps = ps_seg.tile([P, 2 * P], F32, space="PSUM", tag="seg")
                gseg_chunk = gseg_all[:, c * TPC:(c + 1) * TPC]
                for d in range(TPC // 2):
                    job = None
                    if len(p2q) >= LAG:
                        ctx0, c0 = p2q.pop(0)
                        job = (ctx0, kc_by_chunk[c0])
                    ctx = pass1_dt(c, d, seg_ps, gseg_chunk, job)
                    p2q.append((ctx, c))
                kc_by_chunk[c] = stats_chunk(c, seg_ps)
            for ctx0, c0 in p2q:
                pass2_dt(ctx0, kc_by_chunk[c0])

    nc.compile()
    return nc


_BUILD_CACHE = {}


def kernel(target_fea, index, weight, bias):
    from concourse.bass_utils import run_bass_kernel_spmd

    x = np.asarray(target_fea, dtype=np.float32)
    idx = np.asarray(index, dtype=np.int64)
    plans = _plan(idx)
    nchunks = max(len(p[0]) for p in plans)
    consts = _consts(weight, bias)

    cores = [_core_arrays(p, nchunks, x) for p in plans]
    in_maps = []
    for ca in cores:
        m = {"xpad": ca["xpad"], "gseg": ca["gseg"], "rn": ca["rn"],
             "rn1": ca["rn1"]}
        m.update(consts)
        in_maps.append(m)

    if nchunks not in _BUILD_CACHE:
        _BUILD_CACHE[nchunks] = _build(nchunks)
    nc = _BUILD_CACHE[nchunks]

    res = run_bass_kernel_spmd(nc, in_maps, core_ids=list(range(NCORES)))

    out = np.empty((N, F), dtype=np.float32)
    bias_np = np.asarray(bias, dtype=np.float32)
    for c in range(NCORES):
        ca = cores[c]
        out[ca["src_rows"]] = res.results[c]["out"][ca["dst_rows"]]
        for d in ca["n1_dst"]:
            # n==1 segments: reference yields exactly bias
            src = ca["src_rows"][np.searchsorted(ca["dst_rows"], d)]
            out[src] = bias_np
    return out
